# revision 4
# baseline (speedup 1.0000x reference)
# BinaryTreeLSTM forest kernel for 8 trn2 NeuronCores (Bass/Tile).
#
# Strategy: data-parallel over trees. 50 trees are padded to 56 = 8 cores x 7
# trees; each core processes its 7 trees level-by-level (leaves first).
# Within a core, nodes are laid out level-major across its trees so that the
# children of parent column p at level l are exactly columns 2p, 2p+1 of
# level l-1 -- all per-level GEMMs then run on contiguous column ranges.
# Activations live transposed ([feature, node]) so every GEMM keeps the same
# layout: stationary operand = weight tile, moving operand = activation
# columns, PSUM accumulates x-part + children-part + bias (rank-1 matmul).
import numpy as np
import ml_dtypes

TREES, DEPTH, H = 50, 10, 512
PER = 2 ** (DEPTH + 1) - 1          # 2047 nodes per tree
N = TREES * PER                     # 102350
NCORES = 8
TPC = 7                             # trees per core (padded forest: 56 trees)
NLEV = DEPTH + 1
NL = [2 ** (DEPTH - l) for l in range(NLEV)]          # per-tree level sizes
SL = [0]
for _n in NL:
    SL.append(SL[-1] + _n)                            # per-tree level starts
PL = [TPC * n for n in NL]                            # per-core level sizes
LL = [TPC * s for s in SL[:-1]]                       # per-core level col starts
NCOLS = TPC * PER                   # 14329
NPAD = 14336
CH = 256                            # parents per chunk
RES_FROM = 4                        # levels >= RES_FROM read children from SBUF
HB_COLS = LL[RES_FROM - 1] + PL[RES_FROM - 1]         # staged h-bf16 cols (levels < RES_FROM-? )

_BUILT = {}


def _build_kernel():
    """Build + compile the per-core SPMD Bass program (cached)."""
    import concourse.bass as bass  # noqa: F401
    import concourse.mybir as mybir
    import concourse.tile as tile
    from concourse import bacc

    dt = mybir.dt
    Sig = mybir.ActivationFunctionType.Sigmoid
    Tanh = mybir.ActivationFunctionType.Tanh

    nc = bacc.Bacc("TRN2", target_bir_lowering=False, debug=False)

    xT = nc.dram_tensor("xT", [512, NPAD], dt.float16, kind="ExternalInput").ap()
    wiou = nc.dram_tensor("wiou", [512, 1536], dt.float16, kind="ExternalInput").ap()
    ucat = nc.dram_tensor("ucat", [1024, 1536], dt.float16, kind="ExternalInput").ap()
    wf = nc.dram_tensor("wf", [512, 512], dt.float16, kind="ExternalInput").ap()
    ufc = nc.dram_tensor("ufc", [512, 1024], dt.float16, kind="ExternalInput").ap()
    biou = nc.dram_tensor("biou", [1, 1536], dt.float16, kind="ExternalInput").ap()
    bfw = nc.dram_tensor("bfw", [1, 512], dt.float16, kind="ExternalInput").ap()
    hT = nc.dram_tensor("hT", [512, NPAD], dt.float32, kind="ExternalOutput").ap()
    cT = nc.dram_tensor("cT", [512, NPAD], dt.float32, kind="ExternalOutput").ap()
    hb16 = nc.dram_tensor("hb16", [512, HB_COLS], dt.float16).ap()

    xTr = xT.rearrange("(kt kp) n -> kp kt n", kp=128)
    hTr = hT.rearrange("(kt kp) n -> kp kt n", kp=128)
    cTr = cT.rearrange("(kt kp) n -> kp kt n", kp=128)
    hbr = hb16.rearrange("(kt kp) n -> kp kt n", kp=128)

    with tile.TileContext(nc) as tc:
        with (
            tc.tile_pool(name="w", bufs=1) as wp,
            tc.tile_pool(name="persist", bufs=1) as pp,
            tc.tile_pool(name="xin", bufs=3) as xp,
            tc.tile_pool(name="chld", bufs=2) as chp,
            tc.tile_pool(name="gate", bufs=6) as gp,
            tc.tile_pool(name="fg", bufs=5) as fp,
            tc.tile_pool(name="outp", bufs=2) as op,
            tc.tile_pool(name="ps", bufs=4, space="PSUM") as psp,
        ):
            wiou_sb = wp.tile([128, 4, 1536], dt.float16)
            nc.sync.dma_start(wiou_sb[:], wiou.rearrange("(kt kp) m -> kp kt m", kp=128))
            ucat_sb = wp.tile([128, 8, 1536], dt.float16)
            nc.sync.dma_start(ucat_sb[:], ucat.rearrange("(kt kp) m -> kp kt m", kp=128))
            wf_sb = wp.tile([128, 4, 512], dt.float16)
            nc.sync.dma_start(wf_sb[:], wf.rearrange("(kt kp) m -> kp kt m", kp=128))
            ufc_sb = wp.tile([128, 4, 1024], dt.float16)
            nc.sync.dma_start(ufc_sb[:], ufc.rearrange("(kt kp) m -> kp kt m", kp=128))
            biou_sb = wp.tile([1, 1536], dt.float16)
            nc.sync.dma_start(biou_sb[:], biou[:])
            bf_sb = wp.tile([1, 512], dt.float16)
            nc.sync.dma_start(bf_sb[:], bfw[:])
            ones = wp.tile([1, 512], dt.float16)
            nc.vector.memset(ones[:], 1.0)

            # SBUF-resident children h/c for small levels; level l writes slot
            # l % 2, level l+1 reads it. Slot sizes: odd levels (3,5,7,9) write
            # up to PL[3] cols, even (4,6,8) up to PL[4].
            slot_h = [
                pp.tile([128, 4, PL[RES_FROM]], dt.float16, name="sh0"),
                pp.tile([128, 4, PL[RES_FROM - 1]], dt.float16, name="sh1"),
            ]
            slot_c = [
                pp.tile([128, 4, PL[RES_FROM]], dt.float32, name="sc0"),
                pp.tile([128, 4, PL[RES_FROM - 1]], dt.float32, name="sc1"),
            ]

            for l in range(NLEV):
                P = PL[l]
                for c0 in range(0, P, CH):
                    ch = min(CH, P - c0)
                    ch2 = 2 * ch
                    cols = slice(LL[l] + c0, LL[l] + c0 + ch)

                    x_sb = xp.tile([128, 4, CH], dt.float16, tag="x")
                    nc.sync.dma_start(x_sb[:, :, :ch], xTr[:, :, cols])

                    if l > 0:
                        if l < RES_FROM:
                            hch = chp.tile([128, 4, 2 * CH], dt.float16, tag="hch")
                            cch = chp.tile([128, 4, 2 * CH], dt.float32, tag="cch")
                            ccols = slice(LL[l - 1] + 2 * c0, LL[l - 1] + 2 * c0 + ch2)
                            nc.sync.dma_start(hch[:, :, :ch2], hbr[:, :, ccols])
                            nc.sync.dma_start(cch[:, :, :ch2], cTr[:, :, ccols])
                            coff = 0
                        else:
                            hch = slot_h[(l - 1) % 2]
                            cch = slot_c[(l - 1) % 2]
                            coff = 2 * c0

                    # ---- iou = W_iou @ x + U_l @ hL + U_r @ hR + b_iou ----
                    ps_i = psp.tile([128, 4, CH], dt.float32, tag="ps")
                    ps_o = psp.tile([128, 4, CH], dt.float32, tag="ps")
                    ps_u = psp.tile([128, 4, CH], dt.float32, tag="ps")
                    for gg, ps in ((0, ps_i), (4, ps_o), (8, ps_u)):
                        for j in range(4):
                            g = gg + j
                            out = ps[:, j, :ch]
                            gsl = slice(g * 128, (g + 1) * 128)
                            for kt in range(4):
                                nc.tensor.matmul(
                                    out, wiou_sb[:, kt, gsl], x_sb[:, kt, :ch],
                                    start=(kt == 0), stop=False,
                                )
                            if l > 0:
                                for kt in range(8):
                                    if kt < 4:
                                        rhs = hch[:, kt, coff : coff + ch2 : 2]
                                    else:
                                        rhs = hch[:, kt - 4, coff + 1 : coff + ch2 : 2]
                                    nc.tensor.matmul(
                                        out, ucat_sb[:, kt, gsl], rhs,
                                        start=False, stop=False,
                                    )
                            nc.tensor.matmul(
                                out, biou_sb[:1, gsl], ones[:1, :ch],
                                start=False, stop=True,
                            )
                    si = gp.tile([128, 4, CH], dt.float32, tag="g")
                    nc.scalar.activation(si[:, :, :ch], ps_i[:, :, :ch], Sig)
                    so = gp.tile([128, 4, CH], dt.float32, tag="g")
                    nc.scalar.activation(so[:, :, :ch], ps_o[:, :, :ch], Sig)
                    tu = gp.tile([128, 4, CH], dt.float32, tag="g")
                    nc.scalar.activation(tu[:, :, :ch], ps_u[:, :, :ch], Tanh)

                    persist = RES_FROM - 1 <= l <= 9
                    if persist:
                        cn = slot_c[l % 2][:, :, c0 : c0 + ch]
                    else:
                        cn_t = op.tile([128, 4, CH], dt.float32, tag="cn", name="cn")
                        cn = cn_t[:, :, :ch]

                    if l == 0:
                        nc.vector.tensor_mul(out=cn, in0=si[:, :, :ch], in1=tu[:, :, :ch])
                    else:
                        # ---- forget gates over all children, both variants ----
                        # f tile t: variant v=t//2 (0=U_f_l, 1=U_f_r), gates
                        # 2*(t%2)..2*(t%2)+1, each [128, 2, 2ch].
                        f_sb = []
                        for t in range(4):
                            v, g0 = t // 2, 2 * (t % 2)
                            psf = psp.tile([128, 2, 2 * CH], dt.float32, tag="ps")
                            for j in range(2):
                                g = g0 + j
                                out = psf[:, j, :ch2]
                                usl = slice((v * 4 + g) * 128, (v * 4 + g + 1) * 128)
                                for kt in range(4):
                                    nc.tensor.matmul(
                                        out, ufc_sb[:, kt, usl],
                                        hch[:, kt, coff : coff + ch2],
                                        start=(kt == 0), stop=False,
                                    )
                                gsl = slice(g * 128, (g + 1) * 128)
                                for kt in range(4):
                                    xrep = x_sb[:, kt, :ch, None].to_broadcast([128, ch, 2])
                                    nc.tensor.matmul(
                                        out, wf_sb[:, kt, gsl], xrep,
                                        start=False, stop=False,
                                    )
                                nc.tensor.matmul(
                                    out, bf_sb[:1, gsl], ones[:1, :ch2],
                                    start=False, stop=True,
                                )
                            fs = fp.tile([128, 2, 2 * CH], dt.float32, tag="f")
                            nc.scalar.activation(fs[:, :, :ch2], psf[:, :, :ch2], Sig)
                            f_sb.append(fs)
                        # m[g] = (f_l[g] + f_r[g]) * c_child, in place
                        for t in range(2):
                            a, b = f_sb[t], f_sb[t + 2]
                            nc.vector.tensor_add(
                                out=a[:, :, :ch2], in0=a[:, :, :ch2], in1=b[:, :, :ch2]
                            )
                            nc.vector.tensor_mul(
                                out=a[:, :, :ch2], in0=a[:, :, :ch2],
                                in1=cch[:, 2 * t : 2 * t + 2, coff : coff + ch2],
                            )
                        # cn = si*tu + m_left + m_right
                        t1 = op.tile([128, 4, CH], dt.float32, tag="t1")
                        nc.vector.tensor_mul(
                            out=t1[:, :, :ch], in0=si[:, :, :ch], in1=tu[:, :, :ch]
                        )
                        for t in range(2):
                            gsl2 = slice(2 * t, 2 * t + 2)
                            nc.vector.tensor_add(
                                out=cn[:, gsl2],
                                in0=t1[:, gsl2, :ch],
                                in1=f_sb[t][:, :, 0 : ch2 : 2],
                            )
                            nc.vector.tensor_add(
                                out=cn[:, gsl2],
                                in0=cn[:, gsl2],
                                in1=f_sb[t][:, :, 1 : ch2 : 2],
                            )

                    tcn = gp.tile([128, 4, CH], dt.float32, tag="g")
                    nc.scalar.activation(tcn[:, :, :ch], cn, Tanh)
                    hn = op.tile([128, 4, CH], dt.float32, tag="hn")
                    nc.vector.tensor_mul(
                        out=hn[:, :, :ch], in0=so[:, :, :ch], in1=tcn[:, :, :ch]
                    )
                    if persist:
                        nc.vector.tensor_copy(
                            out=slot_h[l % 2][:, :, c0 : c0 + ch], in_=hn[:, :, :ch]
                        )
                    if 0 < l < RES_FROM - 1 or l == 0:
                        hb = op.tile([128, 4, CH], dt.float16, tag="hb")
                        nc.vector.tensor_copy(out=hb[:, :, :ch], in_=hn[:, :, :ch])
                        nc.sync.dma_start(hbr[:, :, cols], hb[:, :, :ch])
                    nc.sync.dma_start(hTr[:, :, cols], hn[:, :, :ch])
                    nc.sync.dma_start(cTr[:, :, cols], cn)

    nc.compile()
    return nc


def _perm():
    """perm[c, j] = global node id for core c's column j (N = zero/dummy)."""
    perm = np.full((NCORES, NPAD), N, np.int64)
    for c in range(NCORES):
        pieces = []
        for l in range(NLEV):
            for t in range(TPC):
                g = c * TPC + t
                if g < TREES:
                    pieces.append(g * PER + SL[l] + np.arange(NL[l], dtype=np.int64))
                else:
                    pieces.append(np.full(NL[l], N, np.int64))
        perm[c, :NCOLS] = np.concatenate(pieces)
    return perm


def kernel(**inputs):
    from concourse.bass_utils import run_bass_kernel_spmd

    if "nc" not in _BUILT:
        _BUILT["nc"] = _build_kernel()
        _BUILT["perm"] = _perm()
    nc = _BUILT["nc"]
    perm = _BUILT["perm"]

    bf16 = np.float16
    f32 = np.float32
    feats = np.asarray(inputs["features"], dtype=f32)
    W_iou = np.asarray(inputs["W_iou_w"], dtype=f32)
    b_iou = np.asarray(inputs["W_iou_b"], dtype=f32)
    U_il = np.asarray(inputs["U_iou_l"], dtype=f32)
    U_ir = np.asarray(inputs["U_iou_r"], dtype=f32)
    W_f = np.asarray(inputs["W_f_w"], dtype=f32)
    b_f = np.asarray(inputs["W_f_b"], dtype=f32)
    U_fl = np.asarray(inputs["U_f_l"], dtype=f32)
    U_fr = np.asarray(inputs["U_f_r"], dtype=f32)

    wshared = {
        "wiou": np.ascontiguousarray(W_iou.T).astype(bf16),
        "ucat": np.ascontiguousarray(
            np.concatenate([U_il.T, U_ir.T], axis=0)
        ).astype(bf16),
        "wf": np.ascontiguousarray(W_f.T).astype(bf16),
        "ufc": np.ascontiguousarray(
            np.concatenate([U_fl.T, U_fr.T], axis=1)
        ).astype(bf16),
        "biou": b_iou[None, :].astype(bf16),
        "bfw": b_f[None, :].astype(bf16),
    }

    fpad = np.concatenate([feats, np.zeros((1, H), f32)], axis=0)
    in_maps = []
    for c in range(NCORES):
        xc = fpad[perm[c]]                       # [NPAD, 512]
        in_maps.append(
            {"xT": np.ascontiguousarray(xc.T).astype(bf16), **wshared}
        )

    _BUILT["in_maps"] = in_maps
    res = run_bass_kernel_spmd(nc, in_maps, list(range(NCORES)))

    h = np.zeros((N, H), f32)
    cc = np.zeros((N, H), f32)
    for c in range(NCORES):
        mask = perm[c] < N
        idx = perm[c][mask]
        h[idx] = res.results[c]["hT"].T[mask]
        cc[idx] = res.results[c]["cT"].T[mask]
    return h, cc


# revision 7
# speedup vs baseline: 21.4159x; 21.4159x over previous
# BinaryTreeLSTM forest kernel for 8 trn2 NeuronCores (Bass/Tile).
#
# Strategy: data-parallel over trees. 50 trees are padded to 56 = 8 cores x 7
# trees; each core processes its 7 trees level-by-level (leaves first).
# Within a core, nodes are laid out level-major across its trees so that the
# children of parent column p at level l are exactly columns 2p, 2p+1 of
# level l-1 -- all per-level GEMMs then run on contiguous column ranges.
# Activations live transposed ([feature, node]) so every GEMM keeps the same
# layout: stationary operand = weight tile, moving operand = activation
# columns, PSUM accumulates x-part + children-part + bias (rank-1 matmul).
import numpy as np
import ml_dtypes

TREES, DEPTH, H = 50, 10, 512
PER = 2 ** (DEPTH + 1) - 1          # 2047 nodes per tree
N = TREES * PER                     # 102350
NCORES = 8
TPC = 7                             # trees per core (padded forest: 56 trees)
NLEV = DEPTH + 1
NL = [2 ** (DEPTH - l) for l in range(NLEV)]          # per-tree level sizes
SL = [0]
for _n in NL:
    SL.append(SL[-1] + _n)                            # per-tree level starts
PL = [TPC * n for n in NL]                            # per-core level sizes
LL = [TPC * s for s in SL[:-1]]                       # per-core level col starts
NCOLS = TPC * PER                   # 14329
NPAD = 14336
CH = 256                            # parents per chunk
RES_FROM = 4                        # levels >= RES_FROM read children from SBUF
HB_COLS = LL[RES_FROM - 1] + PL[RES_FROM - 1]         # staged h-bf16 cols (levels < RES_FROM-? )

_BUILT = {}


def _build_kernel(repeat=1):
    """Build + compile the per-core SPMD Bass program (cached)."""
    import concourse.bass as bass  # noqa: F401
    import concourse.mybir as mybir
    import concourse.tile as tile
    from concourse import bacc

    dt = mybir.dt
    Sig = mybir.ActivationFunctionType.Sigmoid
    Tanh = mybir.ActivationFunctionType.Tanh

    nc = bacc.Bacc("TRN2", target_bir_lowering=False, debug=False)

    xT = nc.dram_tensor("xT", [512, NPAD], dt.float16, kind="ExternalInput").ap()
    wiou = nc.dram_tensor("wiou", [512, 1536], dt.float16, kind="ExternalInput").ap()
    ucat = nc.dram_tensor("ucat", [1024, 1536], dt.float16, kind="ExternalInput").ap()
    wf = nc.dram_tensor("wf", [512, 512], dt.float16, kind="ExternalInput").ap()
    ufc = nc.dram_tensor("ufc", [512, 1024], dt.float16, kind="ExternalInput").ap()
    biou = nc.dram_tensor("biou", [1, 1536], dt.float16, kind="ExternalInput").ap()
    bfw = nc.dram_tensor("bfw", [1, 512], dt.float16, kind="ExternalInput").ap()
    hT = nc.dram_tensor("hT", [512, NPAD], dt.float32, kind="ExternalOutput").ap()
    cT = nc.dram_tensor("cT", [512, NPAD], dt.float32, kind="ExternalOutput").ap()
    hb16 = nc.dram_tensor("hb16", [512, HB_COLS], dt.float16).ap()

    xTr = xT.rearrange("(kt kp) n -> kp kt n", kp=128)
    hTr = hT.rearrange("(kt kp) n -> kp kt n", kp=128)
    cTr = cT.rearrange("(kt kp) n -> kp kt n", kp=128)
    hbr = hb16.rearrange("(kt kp) n -> kp kt n", kp=128)

    with tile.TileContext(nc) as tc:
        with (
            tc.tile_pool(name="w", bufs=1) as wp,
            tc.tile_pool(name="persist", bufs=1) as pp,
            tc.tile_pool(name="xin", bufs=3) as xp,
            tc.tile_pool(name="chld", bufs=2) as chp,
            tc.tile_pool(name="gate", bufs=6) as gp,
            tc.tile_pool(name="fg", bufs=5) as fp,
            tc.tile_pool(name="outp", bufs=2) as op,
            tc.tile_pool(name="ps", bufs=4, space="PSUM") as psp,
        ):
            wiou_sb = wp.tile([128, 4, 1536], dt.float16)
            nc.sync.dma_start(wiou_sb[:], wiou.rearrange("(kt kp) m -> kp kt m", kp=128))
            ucat_sb = wp.tile([128, 8, 1536], dt.float16)
            nc.sync.dma_start(ucat_sb[:], ucat.rearrange("(kt kp) m -> kp kt m", kp=128))
            wf_sb = wp.tile([128, 4, 512], dt.float16)
            nc.sync.dma_start(wf_sb[:], wf.rearrange("(kt kp) m -> kp kt m", kp=128))
            ufc_sb = wp.tile([128, 4, 1024], dt.float16)
            nc.sync.dma_start(ufc_sb[:], ufc.rearrange("(kt kp) m -> kp kt m", kp=128))
            biou_sb = wp.tile([1, 1536], dt.float16)
            nc.sync.dma_start(biou_sb[:], biou[:])
            bf_sb = wp.tile([1, 512], dt.float16)
            nc.sync.dma_start(bf_sb[:], bfw[:])
            ones = wp.tile([1, 512], dt.float16)
            nc.vector.memset(ones[:], 1.0)

            # SBUF-resident children h/c for small levels; level l writes slot
            # l % 2, level l+1 reads it. Slot sizes: odd levels (3,5,7,9) write
            # up to PL[3] cols, even (4,6,8) up to PL[4].
            slot_h = [
                pp.tile([128, 4, PL[RES_FROM]], dt.float16, name="sh0"),
                pp.tile([128, 4, PL[RES_FROM - 1]], dt.float16, name="sh1"),
            ]
            slot_c = [
                pp.tile([128, 4, PL[RES_FROM]], dt.float32, name="sc0"),
                pp.tile([128, 4, PL[RES_FROM - 1]], dt.float32, name="sc1"),
            ]

            import contextlib

            _rep = contextlib.ExitStack()
            if repeat > 1:
                _rep.enter_context(tc.For_i(0, repeat, 1))
            for l in range(NLEV):
                P = PL[l]
                for c0 in range(0, P, CH):
                    ch = min(CH, P - c0)
                    ch2 = 2 * ch
                    cols = slice(LL[l] + c0, LL[l] + c0 + ch)

                    x_sb = xp.tile([128, 4, CH], dt.float16, tag="x")
                    nc.sync.dma_start(x_sb[:, :, :ch], xTr[:, :, cols])

                    if l > 0:
                        if l < RES_FROM:
                            hch = chp.tile([128, 4, 2 * CH], dt.float16, tag="hch")
                            cch = chp.tile([128, 4, 2 * CH], dt.float32, tag="cch")
                            ccols = slice(LL[l - 1] + 2 * c0, LL[l - 1] + 2 * c0 + ch2)
                            nc.sync.dma_start(hch[:, :, :ch2], hbr[:, :, ccols])
                            nc.sync.dma_start(cch[:, :, :ch2], cTr[:, :, ccols])
                            coff = 0
                        else:
                            hch = slot_h[(l - 1) % 2]
                            cch = slot_c[(l - 1) % 2]
                            coff = 2 * c0

                    # ---- iou = W_iou @ x + U_l @ hL + U_r @ hR + b_iou ----
                    ps_i = psp.tile([128, 4, CH], dt.float32, tag="ps")
                    ps_o = psp.tile([128, 4, CH], dt.float32, tag="ps")
                    ps_u = psp.tile([128, 4, CH], dt.float32, tag="ps")
                    for gg, ps in ((0, ps_i), (4, ps_o), (8, ps_u)):
                        for j in range(4):
                            g = gg + j
                            out = ps[:, j, :ch]
                            gsl = slice(g * 128, (g + 1) * 128)
                            for kt in range(4):
                                nc.tensor.matmul(
                                    out, wiou_sb[:, kt, gsl], x_sb[:, kt, :ch],
                                    start=(kt == 0), stop=False,
                                )
                            if l > 0:
                                for kt in range(8):
                                    if kt < 4:
                                        rhs = hch[:, kt, coff : coff + ch2 : 2]
                                    else:
                                        rhs = hch[:, kt - 4, coff + 1 : coff + ch2 : 2]
                                    nc.tensor.matmul(
                                        out, ucat_sb[:, kt, gsl], rhs,
                                        start=False, stop=False,
                                    )
                            nc.tensor.matmul(
                                out, biou_sb[:1, gsl], ones[:1, :ch],
                                start=False, stop=True,
                            )
                    si = gp.tile([128, 4, CH], dt.float32, tag="g")
                    nc.scalar.activation(si[:, :, :ch], ps_i[:, :, :ch], Sig)
                    so = gp.tile([128, 4, CH], dt.float32, tag="g")
                    nc.scalar.activation(so[:, :, :ch], ps_o[:, :, :ch], Sig)
                    tu = gp.tile([128, 4, CH], dt.float32, tag="g")
                    nc.scalar.activation(tu[:, :, :ch], ps_u[:, :, :ch], Tanh)

                    persist = RES_FROM - 1 <= l <= 9
                    if persist:
                        cn = slot_c[l % 2][:, :, c0 : c0 + ch]
                    else:
                        cn_t = op.tile([128, 4, CH], dt.float32, tag="cn", name="cn")
                        cn = cn_t[:, :, :ch]

                    if l == 0:
                        nc.vector.tensor_mul(out=cn, in0=si[:, :, :ch], in1=tu[:, :, :ch])
                    else:
                        # ---- forget gates over all children, both variants ----
                        # f tile t: variant v=t//2 (0=U_f_l, 1=U_f_r), gates
                        # 2*(t%2)..2*(t%2)+1, each [128, 2, 2ch].
                        f_sb = []
                        for t in range(4):
                            v, g0 = t // 2, 2 * (t % 2)
                            psf = psp.tile([128, 2, 2 * CH], dt.float32, tag="ps")
                            for j in range(2):
                                g = g0 + j
                                out = psf[:, j, :ch2]
                                usl = slice((v * 4 + g) * 128, (v * 4 + g + 1) * 128)
                                for kt in range(4):
                                    nc.tensor.matmul(
                                        out, ufc_sb[:, kt, usl],
                                        hch[:, kt, coff : coff + ch2],
                                        start=(kt == 0), stop=False,
                                    )
                                gsl = slice(g * 128, (g + 1) * 128)
                                for kt in range(4):
                                    xrep = x_sb[:, kt, :ch, None].to_broadcast([128, ch, 2])
                                    nc.tensor.matmul(
                                        out, wf_sb[:, kt, gsl], xrep,
                                        start=False, stop=False,
                                    )
                                nc.tensor.matmul(
                                    out, bf_sb[:1, gsl], ones[:1, :ch2],
                                    start=False, stop=True,
                                )
                            fs = fp.tile([128, 2, 2 * CH], dt.float32, tag="f")
                            nc.scalar.activation(fs[:, :, :ch2], psf[:, :, :ch2], Sig)
                            f_sb.append(fs)
                        # m[g] = (f_l[g] + f_r[g]) * c_child, in place
                        for t in range(2):
                            a, b = f_sb[t], f_sb[t + 2]
                            nc.vector.tensor_add(
                                out=a[:, :, :ch2], in0=a[:, :, :ch2], in1=b[:, :, :ch2]
                            )
                            nc.vector.tensor_mul(
                                out=a[:, :, :ch2], in0=a[:, :, :ch2],
                                in1=cch[:, 2 * t : 2 * t + 2, coff : coff + ch2],
                            )
                        # cn = si*tu + m_left + m_right
                        t1 = op.tile([128, 4, CH], dt.float32, tag="t1")
                        nc.vector.tensor_mul(
                            out=t1[:, :, :ch], in0=si[:, :, :ch], in1=tu[:, :, :ch]
                        )
                        for t in range(2):
                            gsl2 = slice(2 * t, 2 * t + 2)
                            nc.vector.tensor_add(
                                out=cn[:, gsl2],
                                in0=t1[:, gsl2, :ch],
                                in1=f_sb[t][:, :, 0 : ch2 : 2],
                            )
                            nc.vector.tensor_add(
                                out=cn[:, gsl2],
                                in0=cn[:, gsl2],
                                in1=f_sb[t][:, :, 1 : ch2 : 2],
                            )

                    tcn = gp.tile([128, 4, CH], dt.float32, tag="g")
                    nc.scalar.activation(tcn[:, :, :ch], cn, Tanh)
                    hn = op.tile([128, 4, CH], dt.float32, tag="hn")
                    nc.vector.tensor_mul(
                        out=hn[:, :, :ch], in0=so[:, :, :ch], in1=tcn[:, :, :ch]
                    )
                    if persist:
                        nc.vector.tensor_copy(
                            out=slot_h[l % 2][:, :, c0 : c0 + ch], in_=hn[:, :, :ch]
                        )
                    if 0 < l < RES_FROM - 1 or l == 0:
                        hb = op.tile([128, 4, CH], dt.float16, tag="hb")
                        nc.vector.tensor_copy(out=hb[:, :, :ch], in_=hn[:, :, :ch])
                        nc.sync.dma_start(hbr[:, :, cols], hb[:, :, :ch])
                    nc.sync.dma_start(hTr[:, :, cols], hn[:, :, :ch])
                    nc.sync.dma_start(cTr[:, :, cols], cn)
            _rep.close()

    nc.compile()
    return nc


def _perm():
    """perm[c, j] = global node id for core c's column j (N = zero/dummy)."""
    perm = np.full((NCORES, NPAD), N, np.int64)
    for c in range(NCORES):
        pieces = []
        for l in range(NLEV):
            for t in range(TPC):
                g = c * TPC + t
                if g < TREES:
                    pieces.append(g * PER + SL[l] + np.arange(NL[l], dtype=np.int64))
                else:
                    pieces.append(np.full(NL[l], N, np.int64))
        perm[c, :NCOLS] = np.concatenate(pieces)
    return perm


def kernel(**inputs):
    from concourse.bass_utils import run_bass_kernel_spmd

    if "nc" not in _BUILT:
        _BUILT["nc"] = _build_kernel()
        _BUILT["perm"] = _perm()
    nc = _BUILT["nc"]
    perm = _BUILT["perm"]

    bf16 = np.float16
    f32 = np.float32
    feats = np.asarray(inputs["features"], dtype=f32)
    W_iou = np.asarray(inputs["W_iou_w"], dtype=f32)
    b_iou = np.asarray(inputs["W_iou_b"], dtype=f32)
    U_il = np.asarray(inputs["U_iou_l"], dtype=f32)
    U_ir = np.asarray(inputs["U_iou_r"], dtype=f32)
    W_f = np.asarray(inputs["W_f_w"], dtype=f32)
    b_f = np.asarray(inputs["W_f_b"], dtype=f32)
    U_fl = np.asarray(inputs["U_f_l"], dtype=f32)
    U_fr = np.asarray(inputs["U_f_r"], dtype=f32)

    wshared = {
        "wiou": np.ascontiguousarray(W_iou.T).astype(bf16),
        "ucat": np.ascontiguousarray(
            np.concatenate([U_il.T, U_ir.T], axis=0)
        ).astype(bf16),
        "wf": np.ascontiguousarray(W_f.T).astype(bf16),
        "ufc": np.ascontiguousarray(
            np.concatenate([U_fl.T, U_fr.T], axis=1)
        ).astype(bf16),
        "biou": b_iou[None, :].astype(bf16),
        "bfw": b_f[None, :].astype(bf16),
    }

    fpad = np.concatenate([feats, np.zeros((1, H), f32)], axis=0)
    in_maps = []
    for c in range(NCORES):
        xc = fpad[perm[c]]                       # [NPAD, 512]
        in_maps.append(
            {"xT": np.ascontiguousarray(xc.T).astype(bf16), **wshared}
        )

    _BUILT["in_maps"] = in_maps
    res = run_bass_kernel_spmd(nc, in_maps, list(range(NCORES)))

    h = np.zeros((N, H), f32)
    cc = np.zeros((N, H), f32)
    for c in range(NCORES):
        mask = perm[c] < N
        idx = perm[c][mask]
        h[idx] = res.results[c]["hT"].T[mask]
        cc[idx] = res.results[c]["cT"].T[mask]
    return h, cc
